# revision 21
# baseline (speedup 1.0000x reference)
"""CSWin attention block Trainium2 kernel.

Sharding: one head per NeuronCore (8 heads / 8 cores). Each core computes
both stripe branches (horizontal + vertical) for its 24 channels, the LePE
depthwise conv + GELU residual, and a partial projection over its 24 z
channels. Host sums the 8 partial projections and adds proj_b.

Kernel math notes:
 - relative-position bias folded into the QK contraction exactly via a
   rank-7 SVD of the 7x7 per-head bias table (contraction 24 -> 31); the
   aug rows are produced by the qkv matmul itself from 7 per-branch
   stripe-phase indicator channels appended to x
 - qkv bias handled via a ones-channel appended to x
 - softmax computed without max subtraction (|scores| < 1 for this problem)
 - softmax denominators come from a ones-column appended to V
 - QK (K=31) runs 2-way row-group packed via tile_position: two j-blocks
   execute concurrently in array rows 0-31 / 32-63, with K+aug replicated
   to partitions 0-30 and Q+aug to 32-62 by SBUF-SBUF DMA
 - per-window softmax normalize: DVE reciprocal of the sums row, DVE
   stream_shuffle partition-broadcast, GpSimd multiply into the padded
   branch image (keeps the PE busy through branch transitions)
 - depthwise 3x3 conv on TensorE as 9 diagonal matmuls over shifted views
   of a zero-padded image tile, row-packed 4 taps at a time (K=128)
"""

import sys

for _p in ("/root/.axon_site/_ro/trn_rl_repo", "/opt/trn_rl_repo"):
    if _p not in sys.path:
        sys.path.append(_p)

import numpy as np
import ml_dtypes

import concourse.bass as bass
import concourse.mybir as mybir
import concourse.tile as tile
from concourse.bass_utils import run_bass_kernel_spmd

BF = ml_dtypes.bfloat16
S = 7
NH = 8
C = 192
HD = C // NH            # 24
SCALE = HD ** -0.5
H = W = 112
NWIN = H // S           # 16
L = S * W               # 784 tokens per window
NCORES = 8
AUG = 31                # 24 qk dims + 7 bias dims
CIN = C + 1 + 2 * S     # x + ones + h-indicators + v-indicators = 207
B0, B1 = 104, 103       # contraction split
QKVW = 88               # q+aug @ 0, k+aug @ 32, v @ 64

F32 = mybir.dt.float32
F16 = mybir.dt.float16
BF16 = mybir.dt.bfloat16


def _split_waits(nc):
    """walrus in this container accepts at most ONE sync wait per
    instruction; hoist extras onto NoOps ahead of the instruction."""
    maxw = 1
    for f in nc.m.functions:
        for bb in f.blocks:
            newlist, changed = [], False
            for inst in bb.instructions:
                si = inst.sync_info
                waits = list(si.on_wait) if si and si.on_wait else []
                if len(waits) > maxw:
                    keep, extra = waits[-maxw:], waits[:-maxw]
                    k = 0
                    while extra:
                        chunk, extra = extra[:maxw], extra[maxw:]
                        newlist.append(mybir.InstNoOp(
                            name=f"{inst.name}-wsplit{k}", engine=inst.engine,
                            ins=[], outs=[],
                            sync_info=mybir.SyncInfo(on_wait=chunk, on_update=[])))
                        k += 1
                    inst.sync_info = mybir.SyncInfo(
                        on_wait=keep,
                        on_update=list(si.on_update) if si.on_update else [])
                    changed = True
                newlist.append(inst)
            if changed:
                bb.instructions = newlist


def build_program(nwin=NWIN, exp_func=None, gelu_func=None, split=True):
    """Build the single-core Bass program (head-agnostic; weights arrive
    pre-sliced per core)."""
    if exp_func is None:
        exp_func = mybir.ActivationFunctionType.Exp
    if gelu_func is None:
        gelu_func = mybir.ActivationFunctionType.Gelu

    nc = bass.Bass()

    d_x = nc.dram_tensor("x_aug", [CIN, H * W], BF16, kind="ExternalInput")
    d_wqkv = {b: nc.dram_tensor(f"wqkv_{b}", [CIN, 128], BF16, kind="ExternalInput")
              for b in ("h", "v")}
    d_eye = nc.dram_tensor("eye88", [QKVW, HD], BF16, kind="ExternalInput")
    d_dw = {b: nc.dram_tensor(f"dwdiag_{b}", [128, 9 * 128], BF16, kind="ExternalInput")
            for b in ("h", "v")}
    d_lepeb = {b: nc.dram_tensor(f"lepeb_{b}", [128, 1], F32, kind="ExternalInput")
               for b in ("h", "v")}
    d_wproj = nc.dram_tensor("wproj", [120, 8 * 96], BF16, kind="ExternalInput")
    d_zeros = nc.dram_tensor("zeros", [1, 4096], BF16, kind="ExternalInput")
    d_out = nc.dram_tensor("out", [C, H * W], F16, kind="ExternalOutput")
    d_recip = {b: nc.dram_tensor(f"recip_scratch_{b}", [nwin, L], F32)
               for b in ("h", "v")}

    with tile.TileContext(nc) as tc:
        import contextlib
        ctx = contextlib.ExitStack()
        with ctx:
            consts = ctx.enter_context(tc.tile_pool(name="consts", bufs=1))
            imgs = ctx.enter_context(tc.tile_pool(name="imgs", bufs=1))

            # ---- persistent constants ----
            # weights FIRST: the DMA queues drain in issue order, so the 300KB
            # of weights must not sit behind the 5.2MB x stream
            wq_sb = {}
            for b in ("h", "v"):
                wq_sb[b] = (consts.tile([B0, 128], BF16, name=f"wq0{b}", tag=f"wq0{b}"),
                            consts.tile([B1, 128], BF16, name=f"wq1{b}", tag=f"wq1{b}"))
                nc.sync.dma_start(out=wq_sb[b][0], in_=d_wqkv[b][0:B0, :])
                nc.sync.dma_start(out=wq_sb[b][1], in_=d_wqkv[b][B0:CIN, :])
            eye_sb = consts.tile([QKVW, HD], BF16, name="eye", tag="eye")
            nc.sync.dma_start(out=eye_sb, in_=d_eye[:, :])
            dw_sb = {b: consts.tile([128, 9 * 128], BF16, name=f"dw{b}", tag=f"dw{b}")
                     for b in ("h", "v")}
            lepeb_sb = {b: consts.tile([128, 1], F32, name=f"lb{b}", tag=f"lb{b}")
                        for b in ("h", "v")}
            for b in ("h", "v"):
                nc.sync.dma_start(out=dw_sb[b], in_=d_dw[b][:, :])
                nc.sync.dma_start(out=lepeb_sb[b], in_=d_lepeb[b][:, :])
            wp_sb = consts.tile([120, 8 * 96], BF16, name="wp", tag="wp")
            nc.sync.dma_start(out=wp_sb, in_=d_wproj[:, :])

            # one tile per 28-row slab so a window's qkv only waits on its own
            # slab's DMA; each slab split into 2 channel-halves x 2 sub-slices
            # so packets spread across more DMA engines
            xv0 = d_x[0:B0].rearrange("c (a b) -> c a b", a=H)
            xv1 = d_x[B0:CIN].rearrange("c (a b) -> c a b", a=H)
            x0s, x1s = [], []
            for sl in range(4):
                rs = slice(28 * sl, 28 * sl + 28)
                t0 = consts.tile([B0, 28, W], BF16, name=f"x0s{sl}", tag=f"x0s{sl}")
                t1 = consts.tile([B1, 28, W], BF16, name=f"x1s{sl}", tag=f"x1s{sl}")
                # split the x stream over two engine DMA queues (each engine
                # owns one HWDGE queue; a single queue serializes the 5.2MB)
                nc.sync.dma_start(out=t0[0:52], in_=xv0[0:52, rs, :])
                nc.sync.dma_start(out=t0[52:B0], in_=xv0[52:B0, rs, :])
                nc.scalar.dma_start(out=t1[0:52], in_=xv1[0:52, rs, :])
                nc.scalar.dma_start(out=t1[52:B1], in_=xv1[52:B1, rs, :])
                x0s.append(t0)
                x1s.append(t1)

            # ---- padded branch images (attention outputs) in flat layout,
            # plus their 4-strip views for the K=128 packed depthwise conv.
            # Only the never-written regions need zeroing (halo ring of pad,
            # 8-partition gaps of pad2): done by DMA from a zeros input so no
            # compute engine burns time on init.
            pad = {b: imgs.tile([HD, H + 2, W + 2], BF16, name=f"pad{b}", tag=f"pad{b}")
                   for b in ("h", "v")}
            pad2 = {b: imgs.tile([128, 30, W + 2], BF16, name=f"pad2{b}", tag=f"pad2{b}")
                    for b in ("h", "v")}
            for b in ("h", "v"):
                # pad halo rows 0 and 113 by DMA (contiguous packets); the
                # column halos are thin strided writes - tiny DVE memsets
                for rsl in (slice(0, 1), slice(H + 1, H + 2)):
                    nc.sync.dma_start(
                        out=pad[b][0:HD, rsl, :],
                        in_=bass.AP(tensor=d_zeros, offset=0,
                                    ap=[[0, HD], [0, 1], [1, W + 2]]))
                for csl in (slice(0, 1), slice(W + 1, W + 2)):
                    nc.vector.memset(pad[b][0:HD, :, csl], 0.0)
                # pad2 partition gaps 24-31 of each 32-group (contiguous)
                for s4 in range(4):
                    nc.sync.dma_start(
                        out=pad2[b][32 * s4 + HD:32 * s4 + 32, :, :],
                        in_=bass.AP(tensor=d_zeros, offset=0,
                                    ap=[[0, 8], [1, 30], [1, W + 2]]))

            # ---- initial PE warm burst on the (early-arriving) weights so
            # the HAM un-throttles while the x stream is still landing
            with tc.tile_pool(name="warm", bufs=1, space="PSUM") as wpool:
                wt = wpool.tile([128, 128], F32, name="wt")
                for _ in range(28):
                    nc.tensor.matmul(wt, wq_sb["h"][0][:, 0:128],
                                     wq_sb["h"][0][:, 0:128],
                                     start=True, stop=True)

            # ================= attention (both branches) =================
            # Per-window pipeline, ACT(exp)-bound by design. PSUM: one shared
            # 3-slot pool (2 banks each) rotates {scores j, qkv-proj pq, V-T
            # pvt} = 9 allocs/window over 3 slots, so each QK j only waits
            # exp(j-3); ps_av holds the AV accumulator (2 banks). QK j runs
            # in array rows 32-63 (K=31, lhs = K+aug rows of qkv_sb in place,
            # rhs = Q+aug replicated to partitions 32-62 by one SBUF DMA).
            # PE issue order puts QK(j+3) BEFORE AV(j) so the psum-slot
            # recycle path (exp -> QK -> exp) is as short as possible.
            for b in ("h", "v"):
                with contextlib.ExitStack() as bctx:
                    work = bctx.enter_context(tc.tile_pool(name=f"work_{b}", bufs=1))
                    epool = bctx.enter_context(tc.tile_pool(name=f"e_{b}", bufs=4))
                    ps_sv = bctx.enter_context(
                        tc.tile_pool(name=f"psq_{b}", bufs=3, space="PSUM"))
                    ps_av = bctx.enter_context(
                        tc.tile_pool(name=f"psav_{b}", bufs=1, space="PSUM"))

                    # double-buffered per-window tiles. qkv/QaR are full-height
                    # [128, L]: the QK contraction runs K=128 (ka rows 32-62
                    # live, everything else times zero rhs rows) because the
                    # PE HAM activity monitor only counts matmuls with >=96
                    # active rows toward the 2.4 GHz un-throttle.
                    qkv_t, QaR_t, va_t, at_t, rbc_t = [], [], [], [], []
                    for i in range(2):
                        qkv_t.append(work.tile([128, L], BF16, name=f"qkv{i}", tag=f"qkv{i}"))
                        nc.vector.memset(qkv_t[i][64:128, :], 0.0)
                        QaR_t.append(work.tile([128, L], BF16, name=f"QaR{i}", tag=f"QaR{i}"))
                        nc.vector.memset(QaR_t[i], 0.0)
                        va_t.append(work.tile([W, S, 32], BF16, name=f"va{i}", tag=f"va{i}"))
                        nc.vector.memset(va_t[i], 0.0)
                        nc.vector.memset(va_t[i][0:W, :, HD:HD + 1], 1.0)
                        at_t.append(work.tile([HD + 1, L], F32, name=f"at{i}", tag=f"at{i}"))
                        rbc_t.append(work.tile([HD, L], F32, name=f"rbc{i}", tag=f"rbc{i}"))
                    eT_t = [epool.tile([W, L], BF16, name=f"eT{i}", tag=f"eT{i}")
                            for i in range(4)]
                    s28_t = [work.tile([28, 28], F32, name=f"s28{i}", tag=f"s28{i}")
                             for i in range(2)]
                    r28_t = [work.tile([28, 28], F32, name=f"r28{i}", tag=f"r28{i}")
                             for i in range(2)]

                    def qkv_stage(w):
                        # qkv+aug projection into a borrowed score slot;
                        # h windows read one 7-row strip of slab w//4; v
                        # windows read a 7-col strip of each of the 4 slabs
                        # (196 px chunks at 256-aligned psum cols so matmuls
                        # stay in-bank)
                        pq = ps_sv.tile([128, 1024], F32, name="pqkv", tag="sv")
                        if b == "h":
                            s, r0 = w // 4, 7 * (w % 4)
                            chunks = [
                                (0, 448, x0s[s][:, r0:r0 + 4, :],
                                 x1s[s][:, r0:r0 + 4, :]),
                                (512, 336, x0s[s][:, r0 + 4:r0 + 7, :],
                                 x1s[s][:, r0 + 4:r0 + 7, :]),
                            ]
                            copies = [(0, 448, 0), (448, 336, 512)]
                        else:
                            cs = slice(7 * w, 7 * w + 7)
                            chunks = [(256 * c, 196, x0s[c][:, :, cs],
                                       x1s[c][:, :, cs]) for c in range(4)]
                            copies = [(196 * c, 196, 256 * c) for c in range(4)]
                        # chunk-outer, K-pass-inner: a start=True matmul resets
                        # has_written bank-wide, so each chunk's accumulation
                        # must complete before the next chunk starts in-bank
                        for col, n, r0c, r1c in chunks:
                            for blk in range(2):
                                nc.tensor.matmul(pq[:, col:col + n], wq_sb[b][blk],
                                                 (r0c, r1c)[blk],
                                                 start=(blk == 0), stop=(blk == 1))
                        qkv_sb = qkv_t[w % 2]
                        for dst, n, src in copies:
                            nc.vector.tensor_copy(out=qkv_sb[0:QKVW, dst:dst + n],
                                                  in_=pq[0:QKVW, src:src + n])
                        # Q+aug replica to partitions 32-62 (QK rhs); issued
                        # on the gpsimd queue so it never sits behind the
                        # bulk x stream on the sync queue
                        nc.gpsimd.dma_start(out=QaR_t[w % 2][32:63, :],
                                            in_=qkv_sb[0:AUG, :])

                    def vt_stage(w):
                        # V transpose into (112, 7, 24) bf16 psum -> vaug
                        qkv_sb, vaug = qkv_t[w % 2], va_t[w % 2]
                        pvt = ps_sv.tile([W, S, HD], BF16, name="pvt", tag="sv")
                        for j in range(S):
                            nc.tensor.transpose(pvt[:, j, :],
                                                qkv_sb[64:88, 112 * j:112 * j + 112],
                                                eye_sb[64:88, :])
                        nc.vector.tensor_copy(out=vaug[0:W, :, 0:HD], in_=pvt)

                    def norm_dst(w):
                        if b == "h":
                            dst = pad[b][0:HD, 1 + 7 * w:8 + 7 * w, 1:1 + W]
                            a = S
                        else:
                            dst = pad[b][0:HD, 1:1 + H, 1 + 7 * w:8 + 7 * w]
                            a = H
                        return dst, a

                    qkv_stage(0)
                    vt_stage(0)

                    for w in range(nwin):
                        qkv_sb, QaR, vaug = (qkv_t[w % 2], QaR_t[w % 2],
                                             va_t[w % 2])
                        pav = ps_av.tile([128, L], F32, name="pav", tag="av")

                        def qk(j):
                            sj = ps_sv.tile([W, L], F32, name="sj", tag="sv")
                            c = 112 * j
                            nc.tensor.matmul(sj[:, 0:512],
                                             qkv_sb[0:128, c:c + 112],
                                             QaR[0:128, 0:512],
                                             start=True, stop=True)
                            nc.tensor.matmul(sj[:, 512:L],
                                             qkv_sb[0:128, c:c + 112],
                                             QaR[0:128, 512:L],
                                             start=True, stop=True)
                            return sj

                        # AV(j) is issued BEFORE QK(j+3): both release on the
                        # exp(j) semaphore, so this order avoids head-of-line
                        # blocking of the psum-slot recycle path on the
                        # in-order PE queue. qkv/vt stages are placed so the
                        # 9-alloc rotation gives every QK a slot freed >= 2
                        # exps earlier.
                        s3 = [qk(0), qk(1), qk(2)]
                        for j in range(S):
                            eT = eT_t[(S * w + j) % 4]
                            nc.scalar.activation(out=eT[0:W, :], in_=s3[j % 3],
                                                 func=exp_func)
                            nc.tensor.matmul(pav[0:HD + 1, 0:512],
                                             vaug[0:W, j, 0:HD + 1],
                                             eT[0:W, 0:512],
                                             start=(j == 0), stop=(j == S - 1))
                            nc.tensor.matmul(pav[0:HD + 1, 512:L],
                                             vaug[0:W, j, 0:HD + 1],
                                             eT[0:W, 512:L],
                                             start=(j == 0), stop=(j == S - 1))
                            if j + 3 < S:
                                s3[j % 3] = qk(j + 3)
                            if j == 2 and w + 1 < nwin:
                                qkv_stage(w + 1)
                            if j == 4 and w + 1 < nwin:
                                vt_stage(w + 1)

                        # per-window normalize, overlapped with the next
                        # window's attention. reciprocal() on DVE is a slow
                        # Newton op (~8 cycles/elem/lane), so the 784 sums are
                        # first reshaped across 28 partitions by DMA (784 ->
                        # 28 elems/lane, ~30x faster recip), then broadcast to
                        # 24 partitions via a DRAM stride-0 bounce; the
                        # multiply runs on GpSimd
                        at, rbc = at_t[w % 2], rbc_t[w % 2]
                        s28, r28 = s28_t[w % 2], r28_t[w % 2]
                        nc.vector.tensor_copy(out=at, in_=pav[0:HD + 1, :])
                        nc.gpsimd.dma_start(out=s28, in_=at[HD:HD + 1, :])
                        nc.vector.reciprocal(out=r28, in_=s28)
                        nc.gpsimd.dma_start(out=d_recip[b][w:w + 1, :], in_=r28)
                        nc.gpsimd.dma_start(
                            out=rbc,
                            in_=bass.AP(tensor=d_recip[b], offset=w * L,
                                        ap=[[0, HD], [1, L]]))
                        dst, a = norm_dst(w)
                        i0 = at[0:HD, :].rearrange("p (a c) -> p a c", a=a)
                        i1 = rbc.rearrange("p (a c) -> p a c", a=a)
                        nc.vector.tensor_tensor(out=dst, in0=i0, in1=i1,
                                                op=mybir.AluOpType.mult)

                    # 4-strip copies: strip s (partitions 32s..32s+24) holds
                    # padded rows 28s..28s+30 of this branch's image, so one
                    # K=128 block-diag matmul covers 4 row-regions at once
                    for s4 in range(4):
                        nc.sync.dma_start(
                            out=pad2[b][32 * s4:32 * s4 + HD, :, :],
                            in_=pad[b][0:HD, 28 * s4:28 * s4 + 30, :])

            # ================= LePE (gelu(dw3x3)+residual) ===============
            # strip-packed: each region j covers rows {28s+4j..+4 | s<4} in
            # one (128, 448) psum tile; 9 diagonal taps accumulate with K=128,
            # gelu lands in strip layout, and the projection contracts each
            # strip block of z with a block-diagonal wp (K=120)
            with contextlib.ExitStack() as lctx:
                gpool = lctx.enter_context(tc.tile_pool(name="g", bufs=1))
                zpool = lctx.enter_context(tc.tile_pool(name="z", bufs=2))
                opool = lctx.enter_context(tc.tile_pool(name="o", bufs=2))

                g2 = {b: gpool.tile([128, S, 448], BF16, name=f"g{b}", tag=f"g{b}")
                      for b in ("h", "v")}
                taps = [(dy, dx) for dy in range(3) for dx in range(3)]
                with tc.tile_pool(name="psdw", bufs=2, space="PSUM") as ps_dw, \
                     tc.tile_pool(name="psp", bufs=2, space="PSUM") as ps_p:
                    # all h regions first: they only need pad2_h, so the PE
                    # stays busy while the v-branch normalize finishes
                    def dw_region(b, j):
                        pdw = ps_dw.tile([128, 448], F32, name="pdw", tag="dw")
                        for t, (dy, dx) in enumerate(taps):
                            rhs = pad2[b][0:128, 4 * j + dy:4 * j + dy + 4,
                                          dx:dx + W]
                            nc.tensor.matmul(
                                pdw, dw_sb[b][:, 128 * t:128 * (t + 1)],
                                rhs, start=(t == 0), stop=(t == 8))
                        nc.scalar.activation(
                            out=g2[b][:, j, :], in_=pdw,
                            func=gelu_func, bias=lepeb_sb[b])

                    for j in range(S):
                        dw_region("h", j)
                    for j in range(S):
                        dw_region("v", j)

                        # combine both branches for region j (all 4 strips)
                        z = zpool.tile([128, 448], BF16, name="z", tag="z")
                        rows = slice(4 * j + 1, 4 * j + 5)
                        nc.vector.tensor_tensor(
                            out=z, in0=pad2["h"][0:128, rows, 1:1 + W],
                            in1=g2["h"][:, j, :], op=mybir.AluOpType.add)
                        nc.vector.tensor_tensor(
                            out=z, in0=z, in1=pad2["v"][0:128, rows, 1:1 + W],
                            op=mybir.AluOpType.add)
                        nc.vector.tensor_tensor(
                            out=z, in0=z, in1=g2["v"][:, j, :],
                            op=mybir.AluOpType.add)

                        # projection: per (strip, out-half) with block-diag wp
                        ot = opool.tile([128, 2, 4, 448], F16, name="ot", tag="ot")
                        for hf in range(2):
                            for s4 in range(4):
                                pp = ps_p.tile([96, 448], F32, name="pp", tag="pp")
                                nc.tensor.matmul(
                                    pp, wp_sb[0:120, (4 * hf + s4) * 96:
                                              (4 * hf + s4 + 1) * 96],
                                    z[0:120, :], start=True, stop=True)
                                nc.vector.tensor_copy(out=ot[0:96, hf, s4, :],
                                                      in_=pp)
                            nc.sync.dma_start(
                                out=bass.AP(tensor=d_out,
                                            offset=96 * hf * (H * W) + 4 * j * W,
                                            ap=[[H * W, 96], [28 * W, 4],
                                                [1, 448]]),
                                in_=ot[0:96, hf, :, :])

    if split:
        _split_waits(nc)
    return nc


def _rel_idx():
    idx = np.arange(S)
    return idx[:, None] - idx[None, :] + S - 1


def prepare_inputs(inputs):
    """Host-side prep: per-core input maps (head h -> core h)."""
    x = np.asarray(inputs["x"], np.float32)[0].reshape(C, H * W)
    px = np.arange(H * W)
    ind_h = (px // W % S == np.arange(S)[:, None]).astype(np.float32)
    ind_v = (px % W % S == np.arange(S)[:, None]).astype(np.float32)
    x_aug = np.concatenate(
        [x, np.ones((1, H * W), np.float32), ind_h, ind_v], 0).astype(BF)

    rel = _rel_idx()
    tbl_h = np.asarray(inputs["bias_table_h"], np.float32)
    tbl_v = np.asarray(inputs["bias_table_v"], np.float32)
    bias_tab = {"h": tbl_h[rel, S - 1, :].transpose(2, 0, 1),
                "v": tbl_v[S - 1, :, :][rel].transpose(2, 0, 1)}
    ind_rows = {"h": C + 1, "v": C + 1 + S}

    qkv_w = {"h": np.asarray(inputs["qkv_h_w"], np.float32),
             "v": np.asarray(inputs["qkv_v_w"], np.float32)}
    qkv_b = {"h": np.asarray(inputs["qkv_h_b"], np.float32),
             "v": np.asarray(inputs["qkv_v_b"], np.float32)}
    lepe_w = {"h": np.asarray(inputs["lepe_h_w"], np.float32),
              "v": np.asarray(inputs["lepe_v_w"], np.float32)}
    lepe_b = {"h": np.asarray(inputs["lepe_h_b"], np.float32),
              "v": np.asarray(inputs["lepe_v_b"], np.float32)}
    proj_w = np.asarray(inputs["proj_w"], np.float32)

    eye88 = np.zeros((QKVW, HD), np.float32)
    eye88[64:88] = np.eye(HD)
    taps = [(dy, dx) for dy in range(3) for dx in range(3)]

    in_maps = []
    for head in range(NCORES):
        m = {"x_aug": x_aug, "eye88": eye88.astype(BF),
             "zeros": np.zeros((1, 4096), BF)}
        cs = slice(head * HD, (head + 1) * HD)
        for b in ("h", "v"):
            wa = np.zeros((CIN, 128), np.float32)
            for s3, (c0, scale) in enumerate(
                    [(0, SCALE), (32, 1.0), (64, 1.0)]):
                rows = slice(s3 * C + head * HD, s3 * C + (head + 1) * HD)
                wa[0:C, c0:c0 + HD] = qkv_w[b][rows].T * scale
                wa[C, c0:c0 + HD] = qkv_b[b][rows] * scale
            U, sv, Vt = np.linalg.svd(bias_tab[b][head])
            Aq = (U * np.sqrt(sv)[None, :])            # (S, 7)
            Ak = (Vt * np.sqrt(sv)[:, None])           # (7, S)
            r0 = ind_rows[b]
            wa[r0:r0 + S, 24:31] = Aq                  # aug_q[j,l]=Aq[row(l),j]
            wa[r0:r0 + S, 56:63] = Ak.T                # aug_k[j,m]=Ak[j,row(m)]
            m[f"wqkv_{b}"] = wa.astype(BF)

            # block-diag-of-diagonals: strip i's 24 channels get their own
            # diagonal inside each (128, 128) tap block
            dw = np.zeros((128, 9 * 128), np.float32)
            for t, (dy, dx) in enumerate(taps):
                for i in range(4):
                    r = slice(32 * i, 32 * i + HD)
                    dw[r, 128 * t + 32 * i:128 * t + 32 * i + HD] = np.diag(
                        lepe_w[b][cs, 0, dy, dx])
            m[f"dwdiag_{b}"] = dw.astype(BF)
            lb = np.zeros((128, 1), np.float32)
            for i in range(4):
                lb[32 * i:32 * i + HD, 0] = lepe_b[b][cs]
            m[f"lepeb_{b}"] = lb
        # projection: col block (4*hf + s)*96 holds proj rows 96hf..96hf+96
        # for strip s's 24 channels (rows 32s..32s+24), zeros elsewhere
        wp = np.zeros((120, 8 * 96), np.float32)
        for hf in range(2):
            for s4 in range(4):
                cb = (4 * hf + s4) * 96
                wp[32 * s4:32 * s4 + HD, cb:cb + 96] = \
                    proj_w[96 * hf:96 * hf + 96, cs].T * 0.5
        m["wproj"] = wp.astype(BF)
        in_maps.append(m)
    return in_maps


_NC_CACHE = {}


def get_nc():
    if "nc" not in _NC_CACHE:
        _NC_CACHE["nc"] = build_program()
    return _NC_CACHE["nc"]


def kernel(**inputs):
    nc = get_nc()
    in_maps = prepare_inputs(inputs)
    res = run_bass_kernel_spmd(nc, in_maps, list(range(NCORES)))
    acc = np.zeros((C, H * W), np.float32)
    for r in res.results:
        acc += r["out"].astype(np.float32)
    acc += np.asarray(inputs["proj_b"], np.float32)[:, None]
    return acc.reshape(1, C, H, W).astype(np.float32)


# revision 24
# speedup vs baseline: 1.0422x; 1.0422x over previous
"""CSWin attention block Trainium2 kernel.

Sharding: one head per NeuronCore (8 heads / 8 cores). Each core computes
both stripe branches (horizontal + vertical) for its 24 channels, the LePE
depthwise conv + GELU residual, and a partial projection over its 24 z
channels. Host sums the 8 partial projections and adds proj_b.

Kernel math notes:
 - relative-position bias folded into the QK contraction exactly via a
   rank-7 SVD of the 7x7 per-head bias table (contraction 24 -> 31); the
   aug rows are produced by the qkv matmul itself from 7 per-branch
   stripe-phase indicator channels appended to x
 - qkv bias handled via a ones-channel appended to x
 - softmax computed without max subtraction (|scores| < 1 for this problem)
 - softmax denominators come from a ones-column appended to V
 - QK (K=31) runs 2-way row-group packed via tile_position: two j-blocks
   execute concurrently in array rows 0-31 / 32-63, with K+aug replicated
   to partitions 0-30 and Q+aug to 32-62 by SBUF-SBUF DMA
 - per-window softmax normalize: DVE reciprocal of the sums row, DVE
   stream_shuffle partition-broadcast, GpSimd multiply into the padded
   branch image (keeps the PE busy through branch transitions)
 - depthwise 3x3 conv on TensorE as 9 diagonal matmuls over shifted views
   of a zero-padded image tile, row-packed 4 taps at a time (K=128)
"""

import sys

for _p in ("/root/.axon_site/_ro/trn_rl_repo", "/opt/trn_rl_repo"):
    if _p not in sys.path:
        sys.path.append(_p)

import numpy as np
import ml_dtypes

import concourse.bass as bass
import concourse.mybir as mybir
import concourse.tile as tile
from concourse.bass_utils import run_bass_kernel_spmd

BF = ml_dtypes.bfloat16
S = 7
NH = 8
C = 192
HD = C // NH            # 24
SCALE = HD ** -0.5
H = W = 112
NWIN = H // S           # 16
L = S * W               # 784 tokens per window
NCORES = 8
AUG = 31                # 24 qk dims + 7 bias dims
CIN = C + 1 + 2 * S     # x + ones + h-indicators + v-indicators = 207
B0, B1 = 104, 103       # contraction split
QKVW = 88               # q+aug @ 0, k+aug @ 32, v @ 64

F32 = mybir.dt.float32
F16 = mybir.dt.float16
BF16 = mybir.dt.bfloat16


def _split_waits(nc):
    """walrus in this container accepts at most ONE sync wait per
    instruction; hoist extras onto NoOps ahead of the instruction."""
    maxw = 1
    for f in nc.m.functions:
        for bb in f.blocks:
            newlist, changed = [], False
            for inst in bb.instructions:
                si = inst.sync_info
                waits = list(si.on_wait) if si and si.on_wait else []
                if len(waits) > maxw:
                    keep, extra = waits[-maxw:], waits[:-maxw]
                    k = 0
                    while extra:
                        chunk, extra = extra[:maxw], extra[maxw:]
                        newlist.append(mybir.InstNoOp(
                            name=f"{inst.name}-wsplit{k}", engine=inst.engine,
                            ins=[], outs=[],
                            sync_info=mybir.SyncInfo(on_wait=chunk, on_update=[])))
                        k += 1
                    inst.sync_info = mybir.SyncInfo(
                        on_wait=keep,
                        on_update=list(si.on_update) if si.on_update else [])
                    changed = True
                newlist.append(inst)
            if changed:
                bb.instructions = newlist


def build_program(nwin=NWIN, exp_func=None, gelu_func=None, split=True):
    """Build the single-core Bass program (head-agnostic; weights arrive
    pre-sliced per core)."""
    if exp_func is None:
        exp_func = mybir.ActivationFunctionType.Exp
    if gelu_func is None:
        gelu_func = mybir.ActivationFunctionType.Gelu

    nc = bass.Bass()

    d_x = nc.dram_tensor("x_aug", [CIN, H * W], BF16, kind="ExternalInput")
    d_wqkv = {b: nc.dram_tensor(f"wqkv_{b}", [CIN, 128], BF16, kind="ExternalInput")
              for b in ("h", "v")}
    d_eye = nc.dram_tensor("eye88", [QKVW, HD], BF16, kind="ExternalInput")
    d_dw = {b: nc.dram_tensor(f"dwdiag_{b}", [128, 9 * 128], BF16, kind="ExternalInput")
            for b in ("h", "v")}
    d_lepeb = {b: nc.dram_tensor(f"lepeb_{b}", [128, 1], F32, kind="ExternalInput")
               for b in ("h", "v")}
    d_wproj = nc.dram_tensor("wproj", [120, 8 * 96], BF16, kind="ExternalInput")
    d_zeros = nc.dram_tensor("zeros", [1, 4096], BF16, kind="ExternalInput")
    d_out = nc.dram_tensor("out", [C, H * W], F16, kind="ExternalOutput")
    d_recip = {b: nc.dram_tensor(f"recip_scratch_{b}", [nwin, L], F32)
               for b in ("h", "v")}

    with tile.TileContext(nc) as tc:
        import contextlib
        ctx = contextlib.ExitStack()
        with ctx:
            consts = ctx.enter_context(tc.tile_pool(name="consts", bufs=1))
            imgs = ctx.enter_context(tc.tile_pool(name="imgs", bufs=1))

            # ---- persistent constants ----
            # weights FIRST: the DMA queues drain in issue order, so the 300KB
            # of weights must not sit behind the 5.2MB x stream
            wq_sb = {}
            for b in ("h", "v"):
                wq_sb[b] = (consts.tile([B0, 128], BF16, name=f"wq0{b}", tag=f"wq0{b}"),
                            consts.tile([B1, 128], BF16, name=f"wq1{b}", tag=f"wq1{b}"))
                nc.sync.dma_start(out=wq_sb[b][0], in_=d_wqkv[b][0:B0, :])
                nc.sync.dma_start(out=wq_sb[b][1], in_=d_wqkv[b][B0:CIN, :])
            eye_sb = consts.tile([QKVW, HD], BF16, name="eye", tag="eye")
            nc.sync.dma_start(out=eye_sb, in_=d_eye[:, :])
            dw_sb = {b: consts.tile([128, 9 * 128], BF16, name=f"dw{b}", tag=f"dw{b}")
                     for b in ("h", "v")}
            lepeb_sb = {b: consts.tile([128, 1], F32, name=f"lb{b}", tag=f"lb{b}")
                        for b in ("h", "v")}
            for b in ("h", "v"):
                nc.sync.dma_start(out=dw_sb[b], in_=d_dw[b][:, :])
                nc.sync.dma_start(out=lepeb_sb[b], in_=d_lepeb[b][:, :])
            wp_sb = consts.tile([120, 8 * 96], BF16, name="wp", tag="wp")
            nc.sync.dma_start(out=wp_sb, in_=d_wproj[:, :])

            # one tile per 28-row slab so a window's qkv only waits on its own
            # slab's DMA; each slab split into 2 channel-halves x 2 sub-slices
            # so packets spread across more DMA engines
            xv0 = d_x[0:B0].rearrange("c (a b) -> c a b", a=H)
            xv1 = d_x[B0:CIN].rearrange("c (a b) -> c a b", a=H)
            x0s, x1s = [], []
            for sl in range(4):
                rs = slice(28 * sl, 28 * sl + 28)
                t0 = consts.tile([B0, 28, W], BF16, name=f"x0s{sl}", tag=f"x0s{sl}")
                t1 = consts.tile([B1, 28, W], BF16, name=f"x1s{sl}", tag=f"x1s{sl}")
                # the whole 5.2MB x stream goes on the scalar engine's HWDGE
                # queue: the sync queue must stay shallow for the per-window
                # latency-critical DMAs (QaR replica, recip bounce), and the
                # gpsimd queue is SWDGE (Q7 descriptor gen, ~5us latency)
                nc.scalar.dma_start(out=t0[0:52], in_=xv0[0:52, rs, :])
                nc.scalar.dma_start(out=t0[52:B0], in_=xv0[52:B0, rs, :])
                nc.scalar.dma_start(out=t1[0:52], in_=xv1[0:52, rs, :])
                nc.scalar.dma_start(out=t1[52:B1], in_=xv1[52:B1, rs, :])
                x0s.append(t0)
                x1s.append(t1)

            # ---- padded branch images (attention outputs) in flat layout,
            # plus their 4-strip views for the K=128 packed depthwise conv.
            # Only the never-written regions need zeroing (halo ring of pad,
            # 8-partition gaps of pad2): done by DMA from a zeros input so no
            # compute engine burns time on init.
            pad = {b: imgs.tile([HD, H + 2, W + 2], BF16, name=f"pad{b}", tag=f"pad{b}")
                   for b in ("h", "v")}
            pad2 = {b: imgs.tile([128, 30, W + 2], BF16, name=f"pad2{b}", tag=f"pad2{b}")
                    for b in ("h", "v")}
            for b in ("h", "v"):
                # pad halo rows 0 and 113 by DMA (contiguous packets); the
                # column halos are thin strided writes - tiny DVE memsets
                for rsl in (slice(0, 1), slice(H + 1, H + 2)):
                    nc.sync.dma_start(
                        out=pad[b][0:HD, rsl, :],
                        in_=bass.AP(tensor=d_zeros, offset=0,
                                    ap=[[0, HD], [0, 1], [1, W + 2]]))
                for csl in (slice(0, 1), slice(W + 1, W + 2)):
                    nc.vector.memset(pad[b][0:HD, :, csl], 0.0)
                # pad2 partition gaps 24-31 of each 32-group (contiguous)
                for s4 in range(4):
                    nc.sync.dma_start(
                        out=pad2[b][32 * s4 + HD:32 * s4 + 32, :, :],
                        in_=bass.AP(tensor=d_zeros, offset=0,
                                    ap=[[0, 8], [1, 30], [1, W + 2]]))

            # ---- initial PE warm burst on the (early-arriving) weights so
            # the HAM un-throttles while the x stream is still landing
            with tc.tile_pool(name="warm", bufs=1, space="PSUM") as wpool:
                wt = wpool.tile([128, 128], F32, name="wt")
                for _ in range(28):
                    nc.tensor.matmul(wt, wq_sb["h"][0][:, 0:128],
                                     wq_sb["h"][0][:, 0:128],
                                     start=True, stop=True)

            # ================= attention (both branches) =================
            # Per-window pipeline, ACT(exp)-bound by design. PSUM: one shared
            # 3-slot pool (2 banks each) rotates {scores j, qkv-proj pq, V-T
            # pvt} = 9 allocs/window over 3 slots, so each QK j only waits
            # exp(j-3); ps_av holds the AV accumulator (2 banks). QK j runs
            # in array rows 32-63 (K=31, lhs = K+aug rows of qkv_sb in place,
            # rhs = Q+aug replicated to partitions 32-62 by one SBUF DMA).
            # PE issue order puts QK(j+3) BEFORE AV(j) so the psum-slot
            # recycle path (exp -> QK -> exp) is as short as possible.
            for b in ("h", "v"):
                with contextlib.ExitStack() as bctx:
                    work = bctx.enter_context(tc.tile_pool(name=f"work_{b}", bufs=1))
                    epool = bctx.enter_context(tc.tile_pool(name=f"e_{b}", bufs=4))
                    ps_sv = bctx.enter_context(
                        tc.tile_pool(name=f"psq_{b}", bufs=3, space="PSUM"))
                    ps_av = bctx.enter_context(
                        tc.tile_pool(name=f"psav_{b}", bufs=1, space="PSUM"))

                    # double-buffered per-window tiles. qkv/QaR are full-height
                    # [128, L]: the QK contraction runs K=128 (ka rows 32-62
                    # live, everything else times zero rhs rows) because the
                    # PE HAM activity monitor only counts matmuls with >=96
                    # active rows toward the 2.4 GHz un-throttle.
                    qkv_t, QaR_t, va_t, at_t, rbc_t = [], [], [], [], []
                    for i in range(2):
                        qkv_t.append(work.tile([128, L], BF16, name=f"qkv{i}", tag=f"qkv{i}"))
                        nc.vector.memset(qkv_t[i][64:128, :], 0.0)
                        QaR_t.append(work.tile([128, L], BF16, name=f"QaR{i}", tag=f"QaR{i}"))
                        nc.vector.memset(QaR_t[i], 0.0)
                        va_t.append(work.tile([W, S, 32], BF16, name=f"va{i}", tag=f"va{i}"))
                        nc.vector.memset(va_t[i], 0.0)
                        nc.vector.memset(va_t[i][0:W, :, HD:HD + 1], 1.0)
                        at_t.append(work.tile([HD + 1, L], F32, name=f"at{i}", tag=f"at{i}"))
                        rbc_t.append(work.tile([HD, L], F32, name=f"rbc{i}", tag=f"rbc{i}"))
                    eT_t = [epool.tile([W, L], BF16, name=f"eT{i}", tag=f"eT{i}")
                            for i in range(4)]
                    s28_t = [work.tile([28, 28], F32, name=f"s28{i}", tag=f"s28{i}")
                             for i in range(2)]
                    r28_t = [work.tile([28, 28], F32, name=f"r28{i}", tag=f"r28{i}")
                             for i in range(2)]

                    def qkv_stage(w):
                        # qkv+aug projection into a borrowed score slot;
                        # h windows read one 7-row strip of slab w//4; v
                        # windows read a 7-col strip of each of the 4 slabs
                        # (196 px chunks at 256-aligned psum cols so matmuls
                        # stay in-bank)
                        pq = ps_sv.tile([128, 1024], F32, name="pqkv", tag="sv")
                        if b == "h":
                            s, r0 = w // 4, 7 * (w % 4)
                            chunks = [
                                (0, 448, x0s[s][:, r0:r0 + 4, :],
                                 x1s[s][:, r0:r0 + 4, :]),
                                (512, 336, x0s[s][:, r0 + 4:r0 + 7, :],
                                 x1s[s][:, r0 + 4:r0 + 7, :]),
                            ]
                            copies = [(0, 448, 0), (448, 336, 512)]
                        else:
                            cs = slice(7 * w, 7 * w + 7)
                            chunks = [(256 * c, 196, x0s[c][:, :, cs],
                                       x1s[c][:, :, cs]) for c in range(4)]
                            copies = [(196 * c, 196, 256 * c) for c in range(4)]
                        # chunk-outer, K-pass-inner: a start=True matmul resets
                        # has_written bank-wide, so each chunk's accumulation
                        # must complete before the next chunk starts in-bank
                        for col, n, r0c, r1c in chunks:
                            for blk in range(2):
                                nc.tensor.matmul(pq[:, col:col + n], wq_sb[b][blk],
                                                 (r0c, r1c)[blk],
                                                 start=(blk == 0), stop=(blk == 1))
                        qkv_sb = qkv_t[w % 2]
                        for dst, n, src in copies:
                            nc.vector.tensor_copy(out=qkv_sb[0:QKVW, dst:dst + n],
                                                  in_=pq[0:QKVW, src:src + n])
                        # Q+aug replica to partitions 32-62 (QK rhs); the sync
                        # HWDGE queue is idle during attention (x went to the
                        # scalar queue) so this completes in sub-us
                        nc.sync.dma_start(out=QaR_t[w % 2][32:63, :],
                                          in_=qkv_sb[0:AUG, :])

                    def vt_stage(w):
                        # V transpose into (112, 7, 24) bf16 psum -> vaug
                        qkv_sb, vaug = qkv_t[w % 2], va_t[w % 2]
                        pvt = ps_sv.tile([W, S, HD], BF16, name="pvt", tag="sv")
                        for j in range(S):
                            nc.tensor.transpose(pvt[:, j, :],
                                                qkv_sb[64:88, 112 * j:112 * j + 112],
                                                eye_sb[64:88, :])
                        nc.vector.tensor_copy(out=vaug[0:W, :, 0:HD], in_=pvt)

                    def norm_dst(w):
                        if b == "h":
                            dst = pad[b][0:HD, 1 + 7 * w:8 + 7 * w, 1:1 + W]
                            a = S
                        else:
                            dst = pad[b][0:HD, 1:1 + H, 1 + 7 * w:8 + 7 * w]
                            a = H
                        return dst, a

                    qkv_stage(0)
                    vt_stage(0)

                    for w in range(nwin):
                        qkv_sb, QaR, vaug = (qkv_t[w % 2], QaR_t[w % 2],
                                             va_t[w % 2])
                        pav = ps_av.tile([128, L], F32, name="pav", tag="av")

                        def qk(j):
                            sj = ps_sv.tile([W, L], F32, name="sj", tag="sv")
                            c = 112 * j
                            nc.tensor.matmul(sj[:, 0:512],
                                             qkv_sb[0:128, c:c + 112],
                                             QaR[0:128, 0:512],
                                             start=True, stop=True)
                            nc.tensor.matmul(sj[:, 512:L],
                                             qkv_sb[0:128, c:c + 112],
                                             QaR[0:128, 512:L],
                                             start=True, stop=True)
                            return sj

                        # AV(j) is issued BEFORE QK(j+3): both release on the
                        # exp(j) semaphore, so this order avoids head-of-line
                        # blocking of the psum-slot recycle path on the
                        # in-order PE queue. qkv/vt stages are placed so the
                        # 9-alloc rotation gives every QK a slot freed >= 2
                        # exps earlier.
                        s3 = [qk(0), qk(1), qk(2)]
                        for j in range(S):
                            eT = eT_t[(S * w + j) % 4]
                            nc.scalar.activation(out=eT[0:W, :], in_=s3[j % 3],
                                                 func=exp_func)
                            nc.tensor.matmul(pav[0:HD + 1, 0:512],
                                             vaug[0:W, j, 0:HD + 1],
                                             eT[0:W, 0:512],
                                             start=(j == 0), stop=(j == S - 1))
                            nc.tensor.matmul(pav[0:HD + 1, 512:L],
                                             vaug[0:W, j, 0:HD + 1],
                                             eT[0:W, 512:L],
                                             start=(j == 0), stop=(j == S - 1))
                            if j + 3 < S:
                                s3[j % 3] = qk(j + 3)
                            if j == 2 and w + 1 < nwin:
                                qkv_stage(w + 1)
                            if j == 4 and w + 1 < nwin:
                                vt_stage(w + 1)

                        # per-window normalize, overlapped with the next
                        # window's attention. reciprocal() on DVE is a slow
                        # Newton op (~8 cycles/elem/lane), so the 784 sums are
                        # first reshaped across 28 partitions by DMA (784 ->
                        # 28 elems/lane, ~30x faster recip), then broadcast to
                        # 24 partitions via a DRAM stride-0 bounce; the
                        # multiply runs on GpSimd
                        at, rbc = at_t[w % 2], rbc_t[w % 2]
                        s28, r28 = s28_t[w % 2], r28_t[w % 2]
                        nc.vector.tensor_copy(out=at, in_=pav[0:HD + 1, :])
                        nc.sync.dma_start(out=s28, in_=at[HD:HD + 1, :])
                        nc.vector.reciprocal(out=r28, in_=s28)
                        nc.sync.dma_start(out=d_recip[b][w:w + 1, :], in_=r28)
                        nc.sync.dma_start(
                            out=rbc,
                            in_=bass.AP(tensor=d_recip[b], offset=w * L,
                                        ap=[[0, HD], [1, L]]))
                        dst, a = norm_dst(w)
                        i0 = at[0:HD, :].rearrange("p (a c) -> p a c", a=a)
                        i1 = rbc.rearrange("p (a c) -> p a c", a=a)
                        nc.vector.tensor_tensor(out=dst, in0=i0, in1=i1,
                                                op=mybir.AluOpType.mult)

                    # 4-strip copies: strip s (partitions 32s..32s+24) holds
                    # padded rows 28s..28s+30 of this branch's image, so one
                    # K=128 block-diag matmul covers 4 row-regions at once
                    for s4 in range(4):
                        nc.sync.dma_start(
                            out=pad2[b][32 * s4:32 * s4 + HD, :, :],
                            in_=pad[b][0:HD, 28 * s4:28 * s4 + 30, :])

            # ================= LePE (gelu(dw3x3)+residual) ===============
            # strip-packed: each region j covers rows {28s+4j..+4 | s<4} in
            # one (128, 448) psum tile; 9 diagonal taps accumulate with K=128,
            # gelu lands in strip layout, and the projection contracts each
            # strip block of z with a block-diagonal wp (K=120)
            with contextlib.ExitStack() as lctx:
                gpool = lctx.enter_context(tc.tile_pool(name="g", bufs=1))
                zpool = lctx.enter_context(tc.tile_pool(name="z", bufs=2))
                opool = lctx.enter_context(tc.tile_pool(name="o", bufs=2))

                g2 = {b: gpool.tile([128, S, 448], BF16, name=f"g{b}", tag=f"g{b}")
                      for b in ("h", "v")}
                taps = [(dy, dx) for dy in range(3) for dx in range(3)]
                with tc.tile_pool(name="psdw", bufs=2, space="PSUM") as ps_dw, \
                     tc.tile_pool(name="psp", bufs=2, space="PSUM") as ps_p:
                    # all h regions first: they only need pad2_h, so the PE
                    # stays busy while the v-branch normalize finishes
                    def dw_region(b, j):
                        pdw = ps_dw.tile([128, 448], F32, name="pdw", tag="dw")
                        for t, (dy, dx) in enumerate(taps):
                            rhs = pad2[b][0:128, 4 * j + dy:4 * j + dy + 4,
                                          dx:dx + W]
                            nc.tensor.matmul(
                                pdw, dw_sb[b][:, 128 * t:128 * (t + 1)],
                                rhs, start=(t == 0), stop=(t == 8))
                        nc.scalar.activation(
                            out=g2[b][:, j, :], in_=pdw,
                            func=gelu_func, bias=lepeb_sb[b])

                    for j in range(S):
                        dw_region("h", j)
                    for j in range(S):
                        dw_region("v", j)

                        # combine both branches for region j (all 4 strips)
                        z = zpool.tile([128, 448], BF16, name="z", tag="z")
                        rows = slice(4 * j + 1, 4 * j + 5)
                        nc.vector.tensor_tensor(
                            out=z, in0=pad2["h"][0:128, rows, 1:1 + W],
                            in1=g2["h"][:, j, :], op=mybir.AluOpType.add)
                        nc.vector.tensor_tensor(
                            out=z, in0=z, in1=pad2["v"][0:128, rows, 1:1 + W],
                            op=mybir.AluOpType.add)
                        nc.vector.tensor_tensor(
                            out=z, in0=z, in1=g2["v"][:, j, :],
                            op=mybir.AluOpType.add)

                        # projection: per (strip, out-half) with block-diag wp
                        ot = opool.tile([128, 2, 4, 448], F16, name="ot", tag="ot")
                        for hf in range(2):
                            for s4 in range(4):
                                pp = ps_p.tile([96, 448], F32, name="pp", tag="pp")
                                nc.tensor.matmul(
                                    pp, wp_sb[0:120, (4 * hf + s4) * 96:
                                              (4 * hf + s4 + 1) * 96],
                                    z[0:120, :], start=True, stop=True)
                                nc.vector.tensor_copy(out=ot[0:96, hf, s4, :],
                                                      in_=pp)
                            nc.sync.dma_start(
                                out=bass.AP(tensor=d_out,
                                            offset=96 * hf * (H * W) + 4 * j * W,
                                            ap=[[H * W, 96], [28 * W, 4],
                                                [1, 448]]),
                                in_=ot[0:96, hf, :, :])

    if split:
        _split_waits(nc)
    return nc


def _rel_idx():
    idx = np.arange(S)
    return idx[:, None] - idx[None, :] + S - 1


def prepare_inputs(inputs):
    """Host-side prep: per-core input maps (head h -> core h)."""
    x = np.asarray(inputs["x"], np.float32)[0].reshape(C, H * W)
    px = np.arange(H * W)
    ind_h = (px // W % S == np.arange(S)[:, None]).astype(np.float32)
    ind_v = (px % W % S == np.arange(S)[:, None]).astype(np.float32)
    x_aug = np.concatenate(
        [x, np.ones((1, H * W), np.float32), ind_h, ind_v], 0).astype(BF)

    rel = _rel_idx()
    tbl_h = np.asarray(inputs["bias_table_h"], np.float32)
    tbl_v = np.asarray(inputs["bias_table_v"], np.float32)
    bias_tab = {"h": tbl_h[rel, S - 1, :].transpose(2, 0, 1),
                "v": tbl_v[S - 1, :, :][rel].transpose(2, 0, 1)}
    ind_rows = {"h": C + 1, "v": C + 1 + S}

    qkv_w = {"h": np.asarray(inputs["qkv_h_w"], np.float32),
             "v": np.asarray(inputs["qkv_v_w"], np.float32)}
    qkv_b = {"h": np.asarray(inputs["qkv_h_b"], np.float32),
             "v": np.asarray(inputs["qkv_v_b"], np.float32)}
    lepe_w = {"h": np.asarray(inputs["lepe_h_w"], np.float32),
              "v": np.asarray(inputs["lepe_v_w"], np.float32)}
    lepe_b = {"h": np.asarray(inputs["lepe_h_b"], np.float32),
              "v": np.asarray(inputs["lepe_v_b"], np.float32)}
    proj_w = np.asarray(inputs["proj_w"], np.float32)

    eye88 = np.zeros((QKVW, HD), np.float32)
    eye88[64:88] = np.eye(HD)
    taps = [(dy, dx) for dy in range(3) for dx in range(3)]

    in_maps = []
    for head in range(NCORES):
        m = {"x_aug": x_aug, "eye88": eye88.astype(BF),
             "zeros": np.zeros((1, 4096), BF)}
        cs = slice(head * HD, (head + 1) * HD)
        for b in ("h", "v"):
            wa = np.zeros((CIN, 128), np.float32)
            for s3, (c0, scale) in enumerate(
                    [(0, SCALE), (32, 1.0), (64, 1.0)]):
                rows = slice(s3 * C + head * HD, s3 * C + (head + 1) * HD)
                wa[0:C, c0:c0 + HD] = qkv_w[b][rows].T * scale
                wa[C, c0:c0 + HD] = qkv_b[b][rows] * scale
            U, sv, Vt = np.linalg.svd(bias_tab[b][head])
            Aq = (U * np.sqrt(sv)[None, :])            # (S, 7)
            Ak = (Vt * np.sqrt(sv)[:, None])           # (7, S)
            r0 = ind_rows[b]
            wa[r0:r0 + S, 24:31] = Aq                  # aug_q[j,l]=Aq[row(l),j]
            wa[r0:r0 + S, 56:63] = Ak.T                # aug_k[j,m]=Ak[j,row(m)]
            m[f"wqkv_{b}"] = wa.astype(BF)

            # block-diag-of-diagonals: strip i's 24 channels get their own
            # diagonal inside each (128, 128) tap block
            dw = np.zeros((128, 9 * 128), np.float32)
            for t, (dy, dx) in enumerate(taps):
                for i in range(4):
                    r = slice(32 * i, 32 * i + HD)
                    dw[r, 128 * t + 32 * i:128 * t + 32 * i + HD] = np.diag(
                        lepe_w[b][cs, 0, dy, dx])
            m[f"dwdiag_{b}"] = dw.astype(BF)
            lb = np.zeros((128, 1), np.float32)
            for i in range(4):
                lb[32 * i:32 * i + HD, 0] = lepe_b[b][cs]
            m[f"lepeb_{b}"] = lb
        # projection: col block (4*hf + s)*96 holds proj rows 96hf..96hf+96
        # for strip s's 24 channels (rows 32s..32s+24), zeros elsewhere
        wp = np.zeros((120, 8 * 96), np.float32)
        for hf in range(2):
            for s4 in range(4):
                cb = (4 * hf + s4) * 96
                wp[32 * s4:32 * s4 + HD, cb:cb + 96] = \
                    proj_w[96 * hf:96 * hf + 96, cs].T * 0.5
        m["wproj"] = wp.astype(BF)
        in_maps.append(m)
    return in_maps


_NC_CACHE = {}


def get_nc():
    if "nc" not in _NC_CACHE:
        _NC_CACHE["nc"] = build_program()
    return _NC_CACHE["nc"]


def kernel(**inputs):
    nc = get_nc()
    in_maps = prepare_inputs(inputs)
    res = run_bass_kernel_spmd(nc, in_maps, list(range(NCORES)))
    acc = np.zeros((C, H * W), np.float32)
    for r in res.results:
        acc += r["out"].astype(np.float32)
    acc += np.asarray(inputs["proj_b"], np.float32)[:, None]
    return acc.reshape(1, C, H, W).astype(np.float32)


# revision 27
# speedup vs baseline: 1.1288x; 1.0831x over previous
"""CSWin attention block Trainium2 kernel.

Sharding: one head per NeuronCore (8 heads / 8 cores). Each core computes
both stripe branches (horizontal + vertical) for its 24 channels, the LePE
depthwise conv + GELU residual, and a partial projection over its 24 z
channels. Host sums the 8 partial projections and adds proj_b.

Kernel math notes:
 - relative-position bias folded into the QK contraction exactly via a
   rank-7 SVD of the 7x7 per-head bias table (contraction 24 -> 31); the
   aug rows are produced by the qkv matmul itself from 7 per-branch
   stripe-phase indicator channels appended to x
 - qkv bias handled via a ones-channel appended to x
 - softmax computed without max subtraction (|scores| < 1 for this problem)
 - softmax denominators come from a ones-column appended to V
 - QK (K=31) runs 2-way row-group packed via tile_position: two j-blocks
   execute concurrently in array rows 0-31 / 32-63, with K+aug replicated
   to partitions 0-30 and Q+aug to 32-62 by SBUF-SBUF DMA
 - per-window softmax normalize: DVE reciprocal of the sums row, DVE
   stream_shuffle partition-broadcast, GpSimd multiply into the padded
   branch image (keeps the PE busy through branch transitions)
 - depthwise 3x3 conv on TensorE as 9 diagonal matmuls over shifted views
   of a zero-padded image tile, row-packed 4 taps at a time (K=128)
"""

import sys

for _p in ("/root/.axon_site/_ro/trn_rl_repo", "/opt/trn_rl_repo"):
    if _p not in sys.path:
        sys.path.append(_p)

import numpy as np
import ml_dtypes

import concourse.bass as bass
import concourse.mybir as mybir
import concourse.tile as tile
from concourse.bass_utils import run_bass_kernel_spmd

BF = ml_dtypes.bfloat16
S = 7
NH = 8
C = 192
HD = C // NH            # 24
SCALE = HD ** -0.5
H = W = 112
NWIN = H // S           # 16
L = S * W               # 784 tokens per window
NCORES = 8
AUG = 31                # 24 qk dims + 7 bias dims
CIN = C + 1 + 2 * S     # x + ones + h-indicators + v-indicators = 207
B0, B1 = 104, 103       # contraction split
QKVW = 88               # q+aug @ 0, k+aug @ 32, v @ 64

F32 = mybir.dt.float32
F16 = mybir.dt.float16
BF16 = mybir.dt.bfloat16


def _split_waits(nc):
    """walrus in this container accepts at most ONE sync wait per
    instruction; hoist extras onto NoOps ahead of the instruction."""
    maxw = 1
    for f in nc.m.functions:
        for bb in f.blocks:
            newlist, changed = [], False
            for inst in bb.instructions:
                si = inst.sync_info
                waits = list(si.on_wait) if si and si.on_wait else []
                if len(waits) > maxw:
                    keep, extra = waits[-maxw:], waits[:-maxw]
                    k = 0
                    while extra:
                        chunk, extra = extra[:maxw], extra[maxw:]
                        newlist.append(mybir.InstNoOp(
                            name=f"{inst.name}-wsplit{k}", engine=inst.engine,
                            ins=[], outs=[],
                            sync_info=mybir.SyncInfo(on_wait=chunk, on_update=[])))
                        k += 1
                    inst.sync_info = mybir.SyncInfo(
                        on_wait=keep,
                        on_update=list(si.on_update) if si.on_update else [])
                    changed = True
                newlist.append(inst)
            if changed:
                bb.instructions = newlist


def build_program(nwin=NWIN, exp_func=None, gelu_func=None, split=True):
    """Build the single-core Bass program (head-agnostic; weights arrive
    pre-sliced per core)."""
    if exp_func is None:
        exp_func = mybir.ActivationFunctionType.Exp
    if gelu_func is None:
        gelu_func = mybir.ActivationFunctionType.Gelu

    nc = bass.Bass()

    d_x = nc.dram_tensor("x_aug", [CIN, H * W], BF16, kind="ExternalInput")
    d_wqkv = {b: nc.dram_tensor(f"wqkv_{b}", [CIN, 128], BF16, kind="ExternalInput")
              for b in ("h", "v")}
    d_eye = nc.dram_tensor("eye88", [QKVW, HD], BF16, kind="ExternalInput")
    d_dw = {b: nc.dram_tensor(f"dwdiag_{b}", [128, 9 * 128], BF16, kind="ExternalInput")
            for b in ("h", "v")}
    d_lepeb = {b: nc.dram_tensor(f"lepeb_{b}", [128, 1], F32, kind="ExternalInput")
               for b in ("h", "v")}
    d_wproj = nc.dram_tensor("wproj", [120, 8 * 96], BF16, kind="ExternalInput")
    d_zeros = nc.dram_tensor("zeros", [1, 4096], BF16, kind="ExternalInput")
    d_out = nc.dram_tensor("out", [C, H * W], F16, kind="ExternalOutput")
    d_recip = {b: nc.dram_tensor(f"recip_scratch_{b}", [nwin, L], F32)
               for b in ("h", "v")}

    with tile.TileContext(nc) as tc:
        import contextlib
        ctx = contextlib.ExitStack()
        with ctx:
            consts = ctx.enter_context(tc.tile_pool(name="consts", bufs=1))
            imgs = ctx.enter_context(tc.tile_pool(name="imgs", bufs=1))

            # ---- persistent constants ----
            # weights FIRST: the DMA queues drain in issue order, so the 300KB
            # of weights must not sit behind the 5.2MB x stream
            wq_sb = {}
            for b in ("h", "v"):
                wq_sb[b] = (consts.tile([B0, 128], BF16, name=f"wq0{b}", tag=f"wq0{b}"),
                            consts.tile([B1, 128], BF16, name=f"wq1{b}", tag=f"wq1{b}"))
                nc.sync.dma_start(out=wq_sb[b][0], in_=d_wqkv[b][0:B0, :])
                nc.sync.dma_start(out=wq_sb[b][1], in_=d_wqkv[b][B0:CIN, :])
            eye_sb = consts.tile([QKVW, HD], BF16, name="eye", tag="eye")
            nc.sync.dma_start(out=eye_sb, in_=d_eye[:, :])
            dw_sb = {b: consts.tile([128, 9 * 128], BF16, name=f"dw{b}", tag=f"dw{b}")
                     for b in ("h", "v")}
            lepeb_sb = {b: consts.tile([128, 1], F32, name=f"lb{b}", tag=f"lb{b}")
                        for b in ("h", "v")}
            for b in ("h", "v"):
                nc.sync.dma_start(out=dw_sb[b], in_=d_dw[b][:, :])
                nc.sync.dma_start(out=lepeb_sb[b], in_=d_lepeb[b][:, :])
            wp_sb = consts.tile([120, 8 * 96], BF16, name="wp", tag="wp")
            nc.sync.dma_start(out=wp_sb, in_=d_wproj[:, :])

            # one tile per 28-row slab so a window's qkv only waits on its own
            # slab's DMA; each slab split into 2 channel-halves x 2 sub-slices
            # so packets spread across more DMA engines
            xv0 = d_x[0:B0].rearrange("c (a b) -> c a b", a=H)
            xv1 = d_x[B0:CIN].rearrange("c (a b) -> c a b", a=H)
            x0s, x1s = [], []
            for sl in range(4):
                rs = slice(28 * sl, 28 * sl + 28)
                t0 = consts.tile([B0, 28, W], BF16, name=f"x0s{sl}", tag=f"x0s{sl}")
                t1 = consts.tile([B1, 28, W], BF16, name=f"x1s{sl}", tag=f"x1s{sl}")
                # the bulk x stream goes on the scalar engine's HWDGE queue:
                # the sync queue must stay shallow for the per-window
                # latency-critical DMAs (QaR replica, recip bounce), and the
                # gpsimd queue is SWDGE (Q7 descriptor gen, ~5us latency).
                # slab 0 gates the first window, so its channel-half 0 rides
                # the (briefly idle) sync queue for 2x arrival bandwidth.
                eng0 = nc.sync if sl == 0 else nc.scalar
                eng0.dma_start(out=t0[0:52], in_=xv0[0:52, rs, :])
                eng0.dma_start(out=t0[52:B0], in_=xv0[52:B0, rs, :])
                nc.scalar.dma_start(out=t1[0:52], in_=xv1[0:52, rs, :])
                nc.scalar.dma_start(out=t1[52:B1], in_=xv1[52:B1, rs, :])
                x0s.append(t0)
                x1s.append(t1)

            # ---- padded branch images (attention outputs) in flat layout,
            # plus their 4-strip views for the K=128 packed depthwise conv.
            # Only the never-written regions need zeroing (halo ring of pad,
            # 8-partition gaps of pad2): done by DMA from a zeros input so no
            # compute engine burns time on init.
            pad = {b: imgs.tile([HD, H + 2, W + 2], BF16, name=f"pad{b}", tag=f"pad{b}")
                   for b in ("h", "v")}
            pad2 = {b: imgs.tile([128, 30, W + 2], BF16, name=f"pad2{b}", tag=f"pad2{b}")
                    for b in ("h", "v")}
            for b in ("h", "v"):
                # pad halo rows 0 and 113 by DMA (contiguous packets); the
                # column halos are thin strided writes - tiny DVE memsets
                for rsl in (slice(0, 1), slice(H + 1, H + 2)):
                    nc.sync.dma_start(
                        out=pad[b][0:HD, rsl, :],
                        in_=bass.AP(tensor=d_zeros, offset=0,
                                    ap=[[0, HD], [0, 1], [1, W + 2]]))
                for csl in (slice(0, 1), slice(W + 1, W + 2)):
                    nc.vector.memset(pad[b][0:HD, :, csl], 0.0)
                # pad2 partition gaps 24-31 of each 32-group (contiguous)
                for s4 in range(4):
                    nc.sync.dma_start(
                        out=pad2[b][32 * s4 + HD:32 * s4 + 32, :, :],
                        in_=bass.AP(tensor=d_zeros, offset=0,
                                    ap=[[0, 8], [1, 30], [1, W + 2]]))

            # ---- initial PE warm burst on the (early-arriving) weights so
            # the HAM un-throttles while the x stream is still landing
            with tc.tile_pool(name="warm", bufs=1, space="PSUM") as wpool:
                wt = wpool.tile([128, 128], F32, name="wt")
                for _ in range(28):
                    nc.tensor.matmul(wt, wq_sb["h"][0][:, 0:128],
                                     wq_sb["h"][0][:, 0:128],
                                     start=True, stop=True)

            # ================= attention (both branches) =================
            # Per-window pipeline, ACT(exp)-bound by design. PSUM: one shared
            # 3-slot pool (2 banks each) rotates {scores j, qkv-proj pq, V-T
            # pvt} = 9 allocs/window over 3 slots, so each QK j only waits
            # exp(j-3); ps_av holds the AV accumulator (2 banks). QK j runs
            # in array rows 32-63 (K=31, lhs = K+aug rows of qkv_sb in place,
            # rhs = Q+aug replicated to partitions 32-62 by one SBUF DMA).
            # PE issue order puts QK(j+3) BEFORE AV(j) so the psum-slot
            # recycle path (exp -> QK -> exp) is as short as possible.
            for b in ("h", "v"):
                with contextlib.ExitStack() as bctx:
                    work = bctx.enter_context(tc.tile_pool(name=f"work_{b}", bufs=1))
                    epool = bctx.enter_context(tc.tile_pool(name=f"e_{b}", bufs=4))
                    ps_sv = bctx.enter_context(
                        tc.tile_pool(name=f"psq_{b}", bufs=3, space="PSUM"))
                    ps_av = bctx.enter_context(
                        tc.tile_pool(name=f"psav_{b}", bufs=1, space="PSUM"))

                    # double-buffered per-window tiles. qkv/QaR are full-height
                    # [128, L]: the QK contraction runs K=128 (ka rows 32-62
                    # live, everything else times zero rhs rows) because the
                    # PE HAM activity monitor only counts matmuls with >=96
                    # active rows toward the 2.4 GHz un-throttle.
                    qkv_t, QaR_t, va_t, at_t, rbc_t = [], [], [], [], []
                    for i in range(2):
                        qkv_t.append(work.tile([128, L], BF16, name=f"qkv{i}", tag=f"qkv{i}"))
                        nc.vector.memset(qkv_t[i][64:128, :], 0.0)
                        QaR_t.append(work.tile([128, L], BF16, name=f"QaR{i}", tag=f"QaR{i}"))
                        nc.vector.memset(QaR_t[i], 0.0)
                        va_t.append(work.tile([W, S, 32], BF16, name=f"va{i}", tag=f"va{i}"))
                        nc.vector.memset(va_t[i], 0.0)
                        nc.vector.memset(va_t[i][0:W, :, HD:HD + 1], 1.0)
                        at_t.append(work.tile([HD + 1, L], F32, name=f"at{i}", tag=f"at{i}"))
                        rbc_t.append(work.tile([HD, L], F32, name=f"rbc{i}", tag=f"rbc{i}"))
                    eT_t = [epool.tile([W, L], BF16, name=f"eT{i}", tag=f"eT{i}")
                            for i in range(4)]
                    s28_t = [work.tile([28, 28], F32, name=f"s28{i}", tag=f"s28{i}")
                             for i in range(2)]
                    r28_t = [work.tile([28, 28], F32, name=f"r28{i}", tag=f"r28{i}")
                             for i in range(2)]

                    def qkv_stage(w):
                        # qkv+aug projection into a borrowed score slot;
                        # h windows read one 7-row strip of slab w//4; v
                        # windows read a 7-col strip of each of the 4 slabs
                        # (196 px chunks at 256-aligned psum cols so matmuls
                        # stay in-bank)
                        pq = ps_sv.tile([128, 1024], F32, name="pqkv", tag="sv")
                        if b == "h":
                            s, r0 = w // 4, 7 * (w % 4)
                            chunks = [
                                (0, 448, x0s[s][:, r0:r0 + 4, :],
                                 x1s[s][:, r0:r0 + 4, :]),
                                (512, 336, x0s[s][:, r0 + 4:r0 + 7, :],
                                 x1s[s][:, r0 + 4:r0 + 7, :]),
                            ]
                            copies = [(0, 448, 0), (448, 336, 512)]
                        else:
                            cs = slice(7 * w, 7 * w + 7)
                            chunks = [(256 * c, 196, x0s[c][:, :, cs],
                                       x1s[c][:, :, cs]) for c in range(4)]
                            copies = [(196 * c, 196, 256 * c) for c in range(4)]
                        # chunk-outer, K-pass-inner: a start=True matmul resets
                        # has_written bank-wide, so each chunk's accumulation
                        # must complete before the next chunk starts in-bank
                        for col, n, r0c, r1c in chunks:
                            for blk in range(2):
                                nc.tensor.matmul(pq[:, col:col + n], wq_sb[b][blk],
                                                 (r0c, r1c)[blk],
                                                 start=(blk == 0), stop=(blk == 1))
                        qkv_sb = qkv_t[w % 2]
                        for dst, n, src in copies:
                            nc.vector.tensor_copy(out=qkv_sb[0:QKVW, dst:dst + n],
                                                  in_=pq[0:QKVW, src:src + n])
                        # Q+aug replica to partitions 32-62 (QK rhs); the sync
                        # HWDGE queue is idle during attention (x went to the
                        # scalar queue) so this completes in sub-us
                        nc.sync.dma_start(out=QaR_t[w % 2][32:63, :],
                                          in_=qkv_sb[0:AUG, :])

                    def vt_stage(w):
                        # V transpose into (112, 7, 24) bf16 psum -> vaug
                        qkv_sb, vaug = qkv_t[w % 2], va_t[w % 2]
                        pvt = ps_sv.tile([W, S, HD], BF16, name="pvt", tag="sv")
                        for j in range(S):
                            nc.tensor.transpose(pvt[:, j, :],
                                                qkv_sb[64:88, 112 * j:112 * j + 112],
                                                eye_sb[64:88, :])
                        nc.vector.tensor_copy(out=vaug[0:W, :, 0:HD], in_=pvt)

                    def norm_dst(w):
                        if b == "h":
                            dst = pad[b][0:HD, 1 + 7 * w:8 + 7 * w, 1:1 + W]
                            a = S
                        else:
                            dst = pad[b][0:HD, 1:1 + H, 1 + 7 * w:8 + 7 * w]
                            a = H
                        return dst, a

                    def qk(w, j):
                        # scores for (window w, block j); K padded to 128
                        qkv_sb, QaR = qkv_t[w % 2], QaR_t[w % 2]
                        sj = ps_sv.tile([W, L], F32, name="sj", tag="sv")
                        c = 112 * j
                        nc.tensor.matmul(sj[:, 0:512],
                                         qkv_sb[0:128, c:c + 112],
                                         QaR[0:128, 0:512],
                                         start=True, stop=True)
                        nc.tensor.matmul(sj[:, 512:L],
                                         qkv_sb[0:128, c:c + 112],
                                         QaR[0:128, 512:L],
                                         start=True, stop=True)
                        return sj

                    # Flat (w, j) software pipeline: the QK for global index
                    # g+3 is issued at index g, so window w+1's first three
                    # score matmuls run DURING window w's tail and the exp
                    # stream never sees the window boundary. The 3 psum score
                    # slots rotate so every QK lands in a slot freed >= 2 exps
                    # earlier. AV(g) precedes QK(g+3) on the PE queue (same
                    # release semaphore). qkv/vt prefetch stages sit at j==0
                    # and j==2 so the replica/vaug for w+1 are ready well
                    # before its QKs issue at j>=4.
                    qkv_stage(0)
                    vt_stage(0)
                    s3 = [qk(0, 0), qk(0, 1), qk(0, 2)]
                    for w in range(nwin):
                        vaug = va_t[w % 2]
                        pav = ps_av.tile([128, L], F32, name="pav", tag="av")
                        for j in range(S):
                            g = S * w + j
                            eT = eT_t[g % 4]
                            nc.scalar.activation(out=eT[0:W, :], in_=s3[g % 3],
                                                 func=exp_func)
                            nc.tensor.matmul(pav[0:HD + 1, 0:512],
                                             vaug[0:W, j, 0:HD + 1],
                                             eT[0:W, 0:512],
                                             start=(j == 0), stop=(j == S - 1))
                            nc.tensor.matmul(pav[0:HD + 1, 512:L],
                                             vaug[0:W, j, 0:HD + 1],
                                             eT[0:W, 512:L],
                                             start=(j == 0), stop=(j == S - 1))
                            gn = g + 3
                            if gn < S * nwin:
                                s3[gn % 3] = qk(gn // S, gn % S)
                            if j == 0 and w + 1 < nwin:
                                qkv_stage(w + 1)
                            if j == 2 and w + 1 < nwin:
                                vt_stage(w + 1)

                        # per-window normalize, overlapped with the next
                        # window's attention. reciprocal() on DVE is a slow
                        # Newton op (~8 cycles/elem/lane), so the 784 sums are
                        # first reshaped across 28 partitions by DMA (784 ->
                        # 28 elems/lane, ~30x faster recip), then broadcast to
                        # 24 partitions via a DRAM stride-0 bounce; the
                        # multiply runs on GpSimd
                        at, rbc = at_t[w % 2], rbc_t[w % 2]
                        s28, r28 = s28_t[w % 2], r28_t[w % 2]
                        nc.vector.tensor_copy(out=at, in_=pav[0:HD + 1, :])
                        nc.sync.dma_start(out=s28, in_=at[HD:HD + 1, :])
                        nc.vector.reciprocal(out=r28, in_=s28)
                        nc.sync.dma_start(out=d_recip[b][w:w + 1, :], in_=r28)
                        nc.sync.dma_start(
                            out=rbc,
                            in_=bass.AP(tensor=d_recip[b], offset=w * L,
                                        ap=[[0, HD], [1, L]]))
                        dst, a = norm_dst(w)
                        i0 = at[0:HD, :].rearrange("p (a c) -> p a c", a=a)
                        i1 = rbc.rearrange("p (a c) -> p a c", a=a)
                        nc.gpsimd.tensor_tensor(out=dst, in0=i0, in1=i1,
                                                op=mybir.AluOpType.mult)

                    # 4-strip copies: strip s (partitions 32s..32s+24) holds
                    # padded rows 28s..28s+30 of this branch's image, so one
                    # K=128 block-diag matmul covers 4 row-regions at once
                    for s4 in range(4):
                        nc.sync.dma_start(
                            out=pad2[b][32 * s4:32 * s4 + HD, :, :],
                            in_=pad[b][0:HD, 28 * s4:28 * s4 + 30, :])

            # ================= LePE (gelu(dw3x3)+residual) ===============
            # strip-packed: each region j covers rows {28s+4j..+4 | s<4} in
            # one (128, 448) psum tile; 9 diagonal taps accumulate with K=128,
            # gelu lands in strip layout, and the projection contracts each
            # strip block of z with a block-diagonal wp (K=120)
            with contextlib.ExitStack() as lctx:
                gpool = lctx.enter_context(tc.tile_pool(name="g", bufs=1))
                zpool = lctx.enter_context(tc.tile_pool(name="z", bufs=2))
                opool = lctx.enter_context(tc.tile_pool(name="o", bufs=2))

                g2 = {b: gpool.tile([128, S, 448], BF16, name=f"g{b}", tag=f"g{b}")
                      for b in ("h", "v")}
                taps = [(dy, dx) for dy in range(3) for dx in range(3)]
                with tc.tile_pool(name="psdw", bufs=2, space="PSUM") as ps_dw, \
                     tc.tile_pool(name="psp", bufs=2, space="PSUM") as ps_p:
                    # all h regions first: they only need pad2_h, so the PE
                    # stays busy while the v-branch normalize finishes
                    def dw_region(b, j):
                        pdw = ps_dw.tile([128, 448], F32, name="pdw", tag="dw")
                        for t, (dy, dx) in enumerate(taps):
                            rhs = pad2[b][0:128, 4 * j + dy:4 * j + dy + 4,
                                          dx:dx + W]
                            nc.tensor.matmul(
                                pdw, dw_sb[b][:, 128 * t:128 * (t + 1)],
                                rhs, start=(t == 0), stop=(t == 8))
                        nc.scalar.activation(
                            out=g2[b][:, j, :], in_=pdw,
                            func=gelu_func, bias=lepeb_sb[b])

                    for j in range(S):
                        dw_region("h", j)
                    for j in range(S):
                        dw_region("v", j)

                        # combine both branches for region j (all 4 strips)
                        z = zpool.tile([128, 448], BF16, name="z", tag="z")
                        rows = slice(4 * j + 1, 4 * j + 5)
                        nc.vector.tensor_tensor(
                            out=z, in0=pad2["h"][0:128, rows, 1:1 + W],
                            in1=g2["h"][:, j, :], op=mybir.AluOpType.add)
                        nc.vector.tensor_tensor(
                            out=z, in0=z, in1=pad2["v"][0:128, rows, 1:1 + W],
                            op=mybir.AluOpType.add)
                        nc.vector.tensor_tensor(
                            out=z, in0=z, in1=g2["v"][:, j, :],
                            op=mybir.AluOpType.add)

                        # projection: per (strip, out-half) with block-diag wp
                        ot = opool.tile([128, 2, 4, 448], F16, name="ot", tag="ot")
                        for hf in range(2):
                            for s4 in range(4):
                                pp = ps_p.tile([96, 448], F32, name="pp", tag="pp")
                                nc.tensor.matmul(
                                    pp, wp_sb[0:120, (4 * hf + s4) * 96:
                                              (4 * hf + s4 + 1) * 96],
                                    z[0:120, :], start=True, stop=True)
                                nc.vector.tensor_copy(out=ot[0:96, hf, s4, :],
                                                      in_=pp)
                            nc.sync.dma_start(
                                out=bass.AP(tensor=d_out,
                                            offset=96 * hf * (H * W) + 4 * j * W,
                                            ap=[[H * W, 96], [28 * W, 4],
                                                [1, 448]]),
                                in_=ot[0:96, hf, :, :])

    if split:
        _split_waits(nc)
    return nc


def _rel_idx():
    idx = np.arange(S)
    return idx[:, None] - idx[None, :] + S - 1


def prepare_inputs(inputs):
    """Host-side prep: per-core input maps (head h -> core h)."""
    x = np.asarray(inputs["x"], np.float32)[0].reshape(C, H * W)
    px = np.arange(H * W)
    ind_h = (px // W % S == np.arange(S)[:, None]).astype(np.float32)
    ind_v = (px % W % S == np.arange(S)[:, None]).astype(np.float32)
    x_aug = np.concatenate(
        [x, np.ones((1, H * W), np.float32), ind_h, ind_v], 0).astype(BF)

    rel = _rel_idx()
    tbl_h = np.asarray(inputs["bias_table_h"], np.float32)
    tbl_v = np.asarray(inputs["bias_table_v"], np.float32)
    bias_tab = {"h": tbl_h[rel, S - 1, :].transpose(2, 0, 1),
                "v": tbl_v[S - 1, :, :][rel].transpose(2, 0, 1)}
    ind_rows = {"h": C + 1, "v": C + 1 + S}

    qkv_w = {"h": np.asarray(inputs["qkv_h_w"], np.float32),
             "v": np.asarray(inputs["qkv_v_w"], np.float32)}
    qkv_b = {"h": np.asarray(inputs["qkv_h_b"], np.float32),
             "v": np.asarray(inputs["qkv_v_b"], np.float32)}
    lepe_w = {"h": np.asarray(inputs["lepe_h_w"], np.float32),
              "v": np.asarray(inputs["lepe_v_w"], np.float32)}
    lepe_b = {"h": np.asarray(inputs["lepe_h_b"], np.float32),
              "v": np.asarray(inputs["lepe_v_b"], np.float32)}
    proj_w = np.asarray(inputs["proj_w"], np.float32)

    eye88 = np.zeros((QKVW, HD), np.float32)
    eye88[64:88] = np.eye(HD)
    taps = [(dy, dx) for dy in range(3) for dx in range(3)]

    in_maps = []
    for head in range(NCORES):
        m = {"x_aug": x_aug, "eye88": eye88.astype(BF),
             "zeros": np.zeros((1, 4096), BF)}
        cs = slice(head * HD, (head + 1) * HD)
        for b in ("h", "v"):
            wa = np.zeros((CIN, 128), np.float32)
            for s3, (c0, scale) in enumerate(
                    [(0, SCALE), (32, 1.0), (64, 1.0)]):
                rows = slice(s3 * C + head * HD, s3 * C + (head + 1) * HD)
                wa[0:C, c0:c0 + HD] = qkv_w[b][rows].T * scale
                wa[C, c0:c0 + HD] = qkv_b[b][rows] * scale
            U, sv, Vt = np.linalg.svd(bias_tab[b][head])
            Aq = (U * np.sqrt(sv)[None, :])            # (S, 7)
            Ak = (Vt * np.sqrt(sv)[:, None])           # (7, S)
            r0 = ind_rows[b]
            wa[r0:r0 + S, 24:31] = Aq                  # aug_q[j,l]=Aq[row(l),j]
            wa[r0:r0 + S, 56:63] = Ak.T                # aug_k[j,m]=Ak[j,row(m)]
            m[f"wqkv_{b}"] = wa.astype(BF)

            # block-diag-of-diagonals: strip i's 24 channels get their own
            # diagonal inside each (128, 128) tap block
            dw = np.zeros((128, 9 * 128), np.float32)
            for t, (dy, dx) in enumerate(taps):
                for i in range(4):
                    r = slice(32 * i, 32 * i + HD)
                    dw[r, 128 * t + 32 * i:128 * t + 32 * i + HD] = np.diag(
                        lepe_w[b][cs, 0, dy, dx])
            m[f"dwdiag_{b}"] = dw.astype(BF)
            lb = np.zeros((128, 1), np.float32)
            for i in range(4):
                lb[32 * i:32 * i + HD, 0] = lepe_b[b][cs]
            m[f"lepeb_{b}"] = lb
        # projection: col block (4*hf + s)*96 holds proj rows 96hf..96hf+96
        # for strip s's 24 channels (rows 32s..32s+24), zeros elsewhere
        wp = np.zeros((120, 8 * 96), np.float32)
        for hf in range(2):
            for s4 in range(4):
                cb = (4 * hf + s4) * 96
                wp[32 * s4:32 * s4 + HD, cb:cb + 96] = \
                    proj_w[96 * hf:96 * hf + 96, cs].T * 0.5
        m["wproj"] = wp.astype(BF)
        in_maps.append(m)
    return in_maps


_NC_CACHE = {}


def get_nc():
    if "nc" not in _NC_CACHE:
        _NC_CACHE["nc"] = build_program()
    return _NC_CACHE["nc"]


def kernel(**inputs):
    nc = get_nc()
    in_maps = prepare_inputs(inputs)
    res = run_bass_kernel_spmd(nc, in_maps, list(range(NCORES)))
    acc = np.zeros((C, H * W), np.float32)
    for r in res.results:
        acc += r["out"].astype(np.float32)
    acc += np.asarray(inputs["proj_b"], np.float32)[:, None]
    return acc.reshape(1, C, H, W).astype(np.float32)
